# revision 65
# baseline (speedup 1.0000x reference)
"""Trainium2 Bass kernel for AdaptiveEmbedding T2I sims.

Reference computation (per full batch):
  cap_repr = ragged-mean(cap_embed, lens)                       (Bc, D)
  bn       = batchnorm(img_embed^T) over (Bi, R) per channel d  (Bi, D, R)
  gamma    = MLP_g(cap_repr); beta = MLP_b(cap_repr)            (Bc, D)
  out      = bn * gamma + beta                                  (Bc, Bi, D, R)
  m        = softmax(out * 10, axis=-1)
  img_vec  = l2norm(mean_r(m * out))                            (Bc, Bi, D)
  sims     = einsum('cbd,cd->bc', img_vec, l2norm(cap_repr))    (Bi, Bc)

Device algebra (per caption c):
  softmax weights are invariant to the +beta shift and to any per-(c,d)
  constant factor, so with A = G*gamma*rstd:
     e = exp(A * x)          x = imgT[d, (b,r)]   (raw image, d on partitions)
     S' = sum_r(e*x) / sum_r(e)
     iv = P1*S' + P2         P1 = gamma*rstd, P2 = gamma*cbn + beta
  iv = R * img_vec(un-normalized);  sims = s1 / (sqrt(s2)*sqrt(s3))
  with s1 = sum_d iv*cv, s2 = sum_d iv^2, s3 = sum_d cv^2 (eps terms are
  ~1e-7 relative and dropped).

Sharding: captions (Bc=32) split 4-per-core across 8 cores; img + MLP params
replicated. Per-core output is its 4 sims columns; host concatenates.

v3: bf16 ingest (img/cap/weights shipped bf16: halves DMA bytes+issues),
dc0 BN stats on DVE tensor_tensor_reduce (ScalarE reaches first exps
sooner), iv chain as per-caption 2-op tensor_scalar (fp32 2x_2P), split
s1/s2 PSUM tiles so the epilogue reads everything on partitions 0..CL-1
(no SBUF->SBUF shuffle DMA), 4-ACT epilogue.
"""

import sys

if "/opt/trn_rl_repo" not in sys.path:
    sys.path.insert(0, "/opt/trn_rl_repo")

import os

import numpy as np
import ml_dtypes

BF = ml_dtypes.bfloat16

# bisection toggles (default = new fast path)
V_FATDMA = os.environ.get("V_FATDMA", "1") == "1"
V_STTHEAD = os.environ.get("V_STTHEAD", "0") == "1"
# GpSimd tree offload measured ~45us SLOWER end-to-end (Q7 two-input ops
# run far below DVE on these strided bf16 views); keep off.
V_GPS = os.environ.get("V_GPS", "0") == "1"

# Problem constants (hardcoded per spec)
Bi, R, D, Bc, T, H = 64, 36, 1024, 32, 64, 128
NCORES = 8
CL = Bc // NCORES            # captions per core = 4
BR = Bi * R                  # 2304
P = 128                      # partitions
ND = D // P                  # 8 d-chunks
GAMMA = 10.0
EPS_BN = 1e-5
EPS_L2 = 1e-8

_COMPILED = None             # cached (nc,) so repeat kernel() calls skip rebuild


def _patch_act_tables():
    """Steer the act-table chooser to `natural_log_exp_and_others` (the only
    set with both exp and ln) for every function this kernel uses, so the
    Scalar engine never swaps table sets mid-kernel (~2.7us per swap)."""
    from concourse import bacc, hw_specs, mybir

    if getattr(bacc, "_act_tables_patched", False):
        return
    orig = hw_specs.get_activation_tables
    AF = mybir.ActivationFunctionType
    mine = {AF.Exp, AF.Ln, AF.Copy, AF.Square, AF.Identity, AF.Relu}

    def patched(arch):
        tables = orig(arch)
        for name, funcs in tables.items():
            if name != "natural_log_exp_and_others":
                tables[name] = funcs - mine
        return tables

    bacc.get_activation_tables = patched
    bacc._act_tables_patched = True


def _register_mma_op():
    """Register MUL_MUL_ADD_ANT: out = (in0*in1)*s0 + s1 (3 ALU stages).
    Fuses the softmax-mean divide-multiply and the iv affine into one DVE
    instruction per caption. uops_sha is computed at runtime (it pins the
    lowered table bytes and is independent of the opcode row)."""
    from concourse.dve_ops import DveOp, OPS
    from concourse.dve_spec import Spec, Src0, Src1, C0, C1, lower
    from concourse.dve_uop import DveOpSpec

    for o in OPS:
        if o.name == "MUL_MUL_ADD_ANT":
            return o
    spec = Spec(
        body=(Src0 * Src1) * C0 + C1,
        reference=lambda in0, in1, s0, s1, imm2: (
            in0.astype(np.float32) * in1) * s0 + s1,
    )
    shas = {}
    for ver in ("v3", "v4"):
        uops = lower(spec, ver=ver)
        shas[ver] = DveOpSpec(name="MUL_MUL_ADD_ANT", opcode=0, uops=uops,
                              rd1_en=True).sha(ver)
    op = DveOp("MUL_MUL_ADD_ANT", spec, subdim=False, uops_sha=shas)
    OPS.append(op)
    # the module-level lookup tables are built at import; extend them
    from concourse import dve_ops as _do
    _do.CUSTOM_DVE_SPECS[op.name] = op.spec
    _do._SUB_OPCODE_FOR_NAME[op.name] = (
        _do._CUSTOM_DVE_ROW_BASE + len(OPS) - 1)
    assert _do._SUB_OPCODE_FOR_NAME[op.name] < 0x20
    return op


def _build_graph():
    from concourse import bacc, mybir, tile
    import concourse.bass as bass

    _patch_act_tables()

    F32 = mybir.dt.float32
    BF16 = mybir.dt.bfloat16
    AF = mybir.ActivationFunctionType
    ALU = mybir.AluOpType

    nc = bacc.Bacc("TRN2", target_bir_lowering=False, debug=False,
                   num_devices=NCORES)

    imgT = nc.declare_dram_parameter("imgT", [D, BR], BF16, isOutput=False)
    cap = nc.declare_dram_parameter("cap", [CL * T, D], BF16, isOutput=False)
    wm = nc.declare_dram_parameter("wm", [CL * T, CL], BF16, isOutput=False)
    Wg1 = nc.declare_dram_parameter("Wg1", [D, H], BF16, isOutput=False)
    Wg2 = nc.declare_dram_parameter("Wg2", [H, D], BF16, isOutput=False)
    Wb1 = nc.declare_dram_parameter("Wb1", [D, H], BF16, isOutput=False)
    Wb2 = nc.declare_dram_parameter("Wb2", [H, D], BF16, isOutput=False)
    # bg1 | bb1 | bg2t [ND] | bb2t*G [ND]
    NBP = 2 + 2 * ND
    bias_pack = nc.declare_dram_parameter("bias_pack", [P, NBP], F32,
                                          isOutput=False)
    out_ext = nc.declare_dram_parameter("out", [CL, Bi], F32, isOutput=True)

    with tile.TileContext(nc) as tc:
        with (
            tc.tile_pool(name="xfpool", bufs=3) as xfp,
            tc.tile_pool(name="smallpool", bufs=1) as smallp,
            tc.tile_pool(name="epool", bufs=2) as ep,
            tc.tile_pool(name="wspool", bufs=2) as wsp,
            tc.tile_pool(name="vpool", bufs=2) as vp,
            tc.tile_pool(name="junkpool", bufs=2) as jp,
            tc.tile_pool(name="psum", bufs=3, space=bass.MemorySpace.PSUM) as pp,
            tc.tile_pool(name="psum_acc", bufs=1, space=bass.MemorySpace.PSUM) as ppa,
            tc.tile_pool(name="psum_s", bufs=1, space=bass.MemorySpace.PSUM) as pps,
        ):
            # ---------- loads. DMA issue instructions cost ~630ns EACH on
            # the issuing engine's queue; bf16 payloads let each img chunk
            # ride in 2 fat issues. Wave 1 = what gates the head (img0,
            # cap+wm for crT, gamma-MLP weights); wave 2 trails on SP. -----
            x_t = [None] * ND
            rr = [0]
            # two HW-DGE issue queues (SP + ScalarE) for the head-critical
            # wave-1 loads; GpSimd's SWDGE queue (wave=4) takes the MLP
            # weights so their issue cost never delays img0/cap.
            dma_engs = [nc.sync, nc.scalar]

            def dma(dst, src, wave):
                if wave == 1:
                    eng = dma_engs[rr[0] % 2]
                    rr[0] += 1
                elif wave == 4:
                    eng = nc.gpsimd
                else:
                    eng = nc.sync
                eng.dma_start(dst, src)

            def emit_img_dma(dc, nsplit=2, wave=3):
                xt = xfp.tile([P, BR], BF16, tag="xall")
                x_t[dc] = xt
                w = BR // nsplit
                for k in range(nsplit):
                    dma(xt[:, k * w:(k + 1) * w],
                        imgT[dc * P:(dc + 1) * P, k * w:(k + 1) * w], wave)

            # wave 1. A HWDGE dma_start only engages a few SDMA rings
            # (~85 GB/s measured), but the SWDGE path fans one issue across
            # all 16 engines: img0 rides GpSimd's queue as its FIRST op
            # (~2us descriptor gen + ~1.4us transfer), leaving both HWDGE
            # queues free for the cap/weight loads that gate the MLP chain.
            xt0 = xfp.tile([P, BR], BF16, tag="xall")
            x_t[0] = xt0
            dma(xt0[:], imgT[0:P, :], 4)
            cap_sb = smallp.tile([P, 2, D], BF16)
            wm_sb = smallp.tile([P, 2, CL], BF16)
            capv = cap[:, :].rearrange("(ct p) c -> p ct c", p=P)
            wmv = wm[:, :].rearrange("(ct p) c -> p ct c", p=P)
            if V_FATDMA:
                dma(cap_sb[:, 0, :], capv[:, 0, :], 1)
                dma(cap_sb[:, 1, :], capv[:, 1, :], 1)
                dma(wm_sb[:], wmv[:], 1)
            else:
                for k in range(8):
                    dma(cap_sb[:, :, k * 128:(k + 1) * 128],
                        capv[:, :, k * 128:(k + 1) * 128], 1)
                    if k < 2:
                        dma(wm_sb[:, k, :], wmv[:, k, :], 1)
            wg1_sb = smallp.tile([P, ND, H], BF16)
            wb1_sb = smallp.tile([P, ND, H], BF16)
            wg1v = Wg1[:, :].rearrange("(a p) h -> p a h", p=P)
            wb1v = Wb1[:, :].rearrange("(a p) h -> p a h", p=P)
            wg2_sb = smallp.tile([P, D], BF16)
            wb2_sb = smallp.tile([P, D], BF16)
            biases_sb = smallp.tile([P, NBP], F32)
            # memsets follow img0 on the GpSimd queue (the coef chain needs
            # eps / expb by ~15us)
            eps_bn_sb = smallp.tile([P, 1], F32)
            nc.gpsimd.memset(eps_bn_sb[:], BR * EPS_BN)
            expb_sb = smallp.tile([P, 1], F32)
            nc.gpsimd.memset(expb_sb[:], 0.5 * float(np.log(BR)) +
                             float(np.log(GAMMA)))
            dma(wg1_sb[:], wg1v[:], 1)
            dma(wg2_sb[:], Wg2[:, :], 1)
            dma(biases_sb[:], bias_pack[:, :], 1)
            bg1_sb = biases_sb[:, 0:1]
            bb1_sb = biases_sb[:, 1:2]
            bg2t_sb = biases_sb[:, 2:2 + ND]
            bb2t_sb = biases_sb[:, 2 + ND:2 + 2 * ND]

            # wave 2 (SP queue): later img chunks + beta-MLP weights
            nspl2 = 2 if V_FATDMA else 6
            emit_img_dma(1, nsplit=nspl2, wave=2)
            emit_img_dma(2, nsplit=nspl2, wave=2)
            if V_FATDMA:
                dma(wb1_sb[:], wb1v[:], 2)
                dma(wb2_sb[:], Wb2[:, :], 2)
            else:
                for k in range(4):
                    dma(wb1_sb[:, 2 * k:2 * k + 2, :],
                        wb1v[:, 2 * k:2 * k + 2, :], 2)
                    dma(wb2_sb[:, k * 256:(k + 1) * 256],
                        Wb2[:, k * 256:(k + 1) * 256], 2)

            # masked-column lhsT tiles: col c of lhs_cv[:, dc, c, :] holds
            # cap_repr column c (everything else 0) -> psA row c = s1_c with
            # zero cross-caption contamination; lhs_one[:, c, :] is one-hot
            # col c -> psB row c = s2_c.
            lhs_cv = smallp.tile([P, ND, CL, CL], BF16)
            nc.gpsimd.memset(lhs_cv[:].rearrange("p a b c -> p (a b c)"), 0.0)
            lhs_one = smallp.tile([P, CL, CL], BF16)
            nc.gpsimd.memset(lhs_one[:].rearrange("p a b -> p (a b)"), 0.0)
            for c in range(CL):
                nc.gpsimd.memset(lhs_one[:, c, c:c + 1], 1.0)

            # ---------- BN stats tiles ----------
            sumx = smallp.tile([P, ND], F32)
            sumsq = smallp.tile([P, ND], F32)
            lnv = smallp.tile([P, ND], F32)
            cbn = smallp.tile([P, ND], F32)
            grstd = smallp.tile([P, ND], F32)
            nmean = smallp.tile([P, ND], F32)
            ustat = smallp.tile([P, ND], F32)
            unstat = smallp.tile([P, ND], F32)

            # dc0 stats split across engines so neither serializes the head:
            # sum(x^2) via DVE scalar_tensor_tensor accum (out -> junk),
            # sum(x) via the ScalarE Copy-accum pass, in parallel.
            if V_STTHEAD:
                jnk0 = jp.tile([P, BR], BF16, tag="junk")
                nc.vector.scalar_tensor_tensor(
                    jnk0[:], xt0[:], 1.0, xt0[:], ALU.mult, ALU.mult,
                    accum_out=sumsq[:, 0:1])
                jnk1 = jp.tile([P, BR], BF16, tag="junk")
                nc.scalar.activation(jnk1[:], xt0[:], AF.Copy,
                                     accum_out=sumx[:, 0:1])

            # ---------- cap_repr^T [d, c] (bf16: feeds bf16 matmuls) ------
            crT = smallp.tile([P, ND, CL], BF16)
            for dc in range(ND):
                pcr = pp.tile([P, CL], F32, tag="pcr")
                for ct in range(2):
                    nc.tensor.matmul(pcr[:], cap_sb[:, ct, dc * P:(dc + 1) * P],
                                     wm_sb[:, ct, :],
                                     start=(ct == 0), stop=(ct == 1))
                nc.vector.tensor_copy(crT[:, dc, :], pcr[:])

            # ---------- conditioning MLPs, transposed form. Gamma first
            # (it gates A[0] and the first exps); beta's DVE bias-adds are
            # emitted separately after the dc0 coefficient chain so they
            # never head-of-line block lnv/A on the Vector queue. ----------
            gammaT = smallp.tile([P, ND, CL], F32)
            betaT = smallp.tile([P, ND, CL], F32)

            def emit_mlp(w1s, w2s, b1s, b2s, dstT, tg):
                phT = ppa.tile([H, CL], F32, tag="ph" + tg)
                for dc in range(ND):
                    nc.tensor.matmul(phT[:], w1s[:, dc, :], crT[:, dc, :],
                                     start=(dc == 0), stop=(dc == ND - 1))
                hT = smallp.tile([H, CL], BF16, tag="hT" + tg)
                nc.vector.tensor_scalar(hT[:], phT[:], b1s, 0.0,
                                        op0=ALU.add, op1=ALU.max)
                for dc in range(ND):
                    pg = pp.tile([P, CL], F32, tag="pcr")
                    nc.tensor.matmul(pg[:], w2s[:, dc * P:(dc + 1) * P],
                                     hT[:], start=True, stop=True)
                    nc.vector.tensor_scalar(dstT[:, dc, :], pg[:],
                                            b2s[:, dc:dc + 1], None,
                                            op0=ALU.add)

            emit_mlp(wg1_sb, wg2_sb, bg1_sb, bg2t_sb, gammaT, "g")

            # ---------- A, P2 tiles (filled per-dchunk in main loop) ----
            A = smallp.tile([P, ND, CL], F32)
            P2 = smallp.tile([P, ND, CL], F32)

            # ---------- main loop ----------
            G4 = 8 * Bi   # e/q slabs of all 4 captions in one shared tree

            def tree_reduce(dst, src, s0=0, s1=2 * CL, gps=False):
                """dst[P, ns*Bi] (fp32) = segmented sum over r of slabs
                [s0:s1] of src[P, 8, Bi*R] (bf16, e_c0|q_c0|..|e_c3|q_c3)
                via a binary tree of 2x-mode tensor_tensor adds."""
                s4 = src[:].rearrange("p a (b r) -> p a b r", r=R)[:, s0:s1]
                t16f = wsp.tile([P, G4, 16], BF16, tag="t16")
                t8f = wsp.tile([P, G4, 8], BF16, tag="t8")
                t4f = wsp.tile([P, G4, 4], BF16, tag="t4")
                t4bf = wsp.tile([P, G4, 4], BF16, tag="t4b")
                t2f = wsp.tile([P, G4, 2], BF16, tag="t2")
                gs = slice(s0 * Bi, s1 * Bi)
                t16 = t16f[:, gs]
                t8 = t8f[:, gs]
                t4 = t4f[:, gs]
                t4b = t4bf[:, gs]
                t2 = t2f[:, gs]
                nc.vector.tensor_add(t16[:], s4[:, :, :, 0:16], s4[:, :, :, 16:32])
                nc.vector.tensor_add(t8[:], t16[:, :, 0:8], t16[:, :, 8:16])
                # the last four levels move to the (otherwise idle) GpSimd
                # engine: ~2x slower per element there, but it runs in the
                # shadow of the DVE's next big ops. Kept on DVE for the last
                # chunk, where the tree sits on the tail critical path.
                eng = nc.gpsimd if gps else nc.vector
                eng.tensor_add(t4[:], t8[:, :, 0:4], t8[:, :, 4:8])
                eng.tensor_add(
                    t4b[:], t4[:],
                    s4[:, :, :, 32:36].rearrange("p a b r -> p (a b) r"))
                eng.tensor_add(t2[:], t4b[:, :, 0:2], t4b[:, :, 2:4])
                eng.tensor_add(
                    dst[:, s0 * Bi:s1 * Bi].rearrange("p (g o) -> p g o", o=1),
                    t2[:, :, 0:1], t2[:, :, 1:2])

            psA = pps.tile([CL, 2 * Bi], F32, tag="sA")   # row c: [s1_c | .]
            psB = ppa.tile([CL, 2 * Bi], F32, tag="sB")   # row c: [. | s2_c]
            ps3 = ppa.tile([CL, 1], F32, tag="s3")
            lns3 = smallp.tile([CL, 1], F32)
            mh3 = smallp.tile([CL, 1], F32)

            def emit_stats_acc(dc):
                """The two ScalarE accumulate passes for chunk dc (outputs
                discarded; only the accumulators matter)."""
                d1 = dc + 1
                ja = jp.tile([P, BR], BF16, tag="junk")
                nc.scalar.activation(ja[:], x_t[dc][:], AF.Copy,
                                     accum_out=sumx[:, dc:d1])
                jb = jp.tile([P, BR], BF16, tag="junk")
                nc.scalar.activation(jb[:], x_t[dc][:], AF.Square,
                                     accum_out=sumsq[:, dc:d1])

            def emit_stats_coef(dc):
                """BN coefficient chain for chunk dc: grstd = G*rstd =
                exp(-0.5*ln(BR*var + BR*eps) + 0.5*ln(BR) + ln(G)) with
                BR*var = sumsq - (sumx/sqrt(BR))^2. The GAMMA on iv is
                harmless: sims is invariant to uniform iv scaling, and the
                host pre-scales Wb2/bb2 by GAMMA to match.

                dc0 splits the chain across DVE+SE (shortest head latency);
                dc>=1 runs it entirely on ScalarE, which has slack while the
                DVE binds — this drops four DVE ops and ~4 cross-engine
                semaphore hops per chunk, and makes the exps' A dependency
                same-queue."""
                d1 = dc + 1
                if dc == 0:
                    nc.vector.tensor_scalar_mul(nmean[:, dc:d1],
                                                sumx[:, dc:d1], -1.0 / BR)
                    nc.vector.scalar_tensor_tensor(lnv[:, dc:d1],
                                                   sumx[:, dc:d1],
                                                   nmean[:, dc:d1],
                                                   sumsq[:, dc:d1],
                                                   ALU.mult, ALU.add)
                    nc.scalar.activation(lnv[:, dc:d1], lnv[:, dc:d1], AF.Ln,
                                         bias=eps_bn_sb[:])
                    nc.scalar.activation(grstd[:, dc:d1], lnv[:, dc:d1],
                                         AF.Exp, scale=-0.5, bias=expb_sb[:])
                    nc.vector.tensor_mul(cbn[:, dc:d1], nmean[:, dc:d1],
                                         grstd[:, dc:d1])
                    nc.vector.tensor_scalar_mul(A[:, dc, :], gammaT[:, dc, :],
                                                grstd[:, dc:d1])
                    return
                nc.scalar.activation(ustat[:, dc:d1], sumx[:, dc:d1],
                                     AF.Square, scale=1.0 / 48.0)
                nc.scalar.activation(unstat[:, dc:d1], ustat[:, dc:d1],
                                     AF.Identity, scale=-1.0,
                                     bias=eps_bn_sb[:])
                nc.scalar.activation(lnv[:, dc:d1], sumsq[:, dc:d1], AF.Ln,
                                     bias=unstat[:, dc:d1])
                nc.scalar.activation(grstd[:, dc:d1], lnv[:, dc:d1], AF.Exp,
                                     scale=-0.5, bias=expb_sb[:])
                nc.scalar.activation(nmean[:, dc:d1], sumx[:, dc:d1],
                                     AF.Copy, scale=-1.0 / BR)
                nc.scalar.activation(cbn[:, dc:d1], nmean[:, dc:d1],
                                     AF.Copy, scale=grstd[:, dc:d1])
                nc.scalar.activation(A[:, dc, :], gammaT[:, dc, :],
                                     AF.Copy, scale=grstd[:, dc:d1])

            def emit_p2(dc):
                """P2 = gammaT*cbn + betaT as one DVE scalar_tensor_tensor."""
                nc.vector.scalar_tensor_tensor(P2[:, dc, :], gammaT[:, dc, :],
                                               cbn[:, dc:dc + 1],
                                               betaT[:, dc, :],
                                               ALU.mult, ALU.add)

            # ---------- s3[c] = sum_d cv^2 ----------
            # emitted mid-main-loop: late enough that the scheduler cannot
            # hoist these 32 PE matmuls ahead of the MLP chain, early
            # enough that ps3 is long done when the epilogue needs it
            def emit_s3():
                for c in range(CL):
                    for dcc in range(ND):
                        nc.tensor.matmul(ps3[:], lhs_cv[:, dcc, c, :],
                                         crT[:, dcc, c:c + 1],
                                         start=(c == 0 and dcc == 0),
                                         stop=(c == CL - 1 and dcc == ND - 1))

            if not V_STTHEAD:
                emit_stats_acc(0)
            emit_stats_coef(0)
            # beta MLP + dc0's P2 after the dc0 coefficient chain: its DVE
            # bias-adds then sit behind lnv/A in the Vector queue, and the
            # late-arriving wave-2 beta weights can't stall anything early.
            emit_mlp(wb1_sb, wb2_sb, bb1_sb, bb2t_sb, betaT, "b")
            emit_p2(0)
            # cap_repr diagonals into the masked lhsT
            for c in range(CL):
                nc.vector.tensor_copy(lhs_cv[:, :, c, c:c + 1],
                                      crT[:, :, c:c + 1])
            for dc in range(ND):
                if dc + 3 < ND:
                    emit_img_dma(dc + 3)
                if dc == 2:
                    emit_s3()
                eq = ep.tile([P, 2 * CL, BR], BF16, tag="eq")
                eqv = eq[:].rearrange("p (a s) n -> p a s n", s=2)
                xb = x_t[dc][:].unsqueeze(1)
                # dc0 and the last dc run in caption-pair halves: dc0 so the
                # first tree starts after just two exps (shorter ramp), the
                # last so iv/psum matmuls of half 1 overlap half 2's tree
                # (shorter tail).
                halves = dc == 0 or dc == ND - 1

                for c in range(CL):
                    nc.scalar.activation(eq[:, 2 * c, :], x_t[dc][:],
                                         AF.Exp,
                                         scale=A[:, dc, c:c + 1])
                # next chunk's accum passes AFTER all four exps: the second
                # q-mul (which waits on exp c3) isn't pushed back by 4.4us
                # of stats sitting mid-exps on the ScalarE queue.
                if dc + 1 < ND:
                    emit_stats_acc(dc + 1)

                sesq = wsp.tile([P, G4], F32, tag="sesq")
                s4v = sesq[:].rearrange("p (a e b) -> p a e b", a=CL, e=2)
                rec = wsp.tile([P, CL, Bi], F32, tag="rec")
                Sp = wsp.tile([P, CL, Bi], F32, tag="Sp")
                ivv = vp.tile([P, 2, CL, Bi], BF16, tag="ivv")

                def emit_mul(hh):
                    nc.vector.tensor_mul(
                        eqv[:, 2 * hh:2 * hh + 2, 1, :],
                        eqv[:, 2 * hh:2 * hh + 2, 0, :],
                        xb.broadcast_to([P, 2, BR]))

                def emit_post(hh=None):
                    """rec/Sp/iv + iv^2 + psum matmuls, for caption pair hh
                    (hh=None: all four captions in full-width ops). (A fused
                    (q*rec)*A+P2 custom-DVE op measured ~1us slower overall:
                    4 small 1x custom ops cost more than 2 wide 2x ops.)"""
                    cs = slice(0, CL) if hh is None else slice(2 * hh,
                                                               2 * hh + 2)
                    ncap = cs.stop - cs.start
                    nc.vector.reciprocal_approx_fast(rec[:, cs, :],
                                                     s4v[:, cs, 0, :])
                    nc.vector.tensor_mul(Sp[:, cs, :], s4v[:, cs, 1, :],
                                         rec[:, cs, :])
                    Ab = A[:, dc, cs].unsqueeze(2).broadcast_to([P, ncap, Bi])
                    P2b = P2[:, dc, cs].unsqueeze(2).broadcast_to(
                        [P, ncap, Bi])
                    nc.vector.tensor_mul(ivv[:, 0, cs, :], Sp[:, cs, :], Ab)
                    nc.vector.tensor_add(ivv[:, 0, cs, :], ivv[:, 0, cs, :],
                                         P2b)
                    nc.vector.tensor_mul(ivv[:, 1, cs, :], ivv[:, 0, cs, :],
                                         ivv[:, 0, cs, :])
                    for c in range(cs.start, cs.stop):
                        nc.tensor.matmul(
                            psA[:], lhs_cv[:, dc, c, :], ivv[:, :, c, :],
                            start=(dc == 0 and c == 0),
                            stop=(dc == ND - 1 and c == CL - 1))
                        nc.tensor.matmul(
                            psB[:], lhs_one[:, c, :], ivv[:, :, c, :],
                            start=(dc == 0 and c == 0),
                            stop=(dc == ND - 1 and c == CL - 1))

                gps = V_GPS and dc < ND - 1
                if halves:
                    emit_mul(0)
                    tree_reduce(sesq, eq, 0, CL, gps)
                    emit_mul(1)
                    if dc == ND - 1:
                        emit_post(0)
                    if dc + 1 < ND:
                        emit_stats_coef(dc + 1)
                        emit_p2(dc + 1)
                    tree_reduce(sesq, eq, CL, 2 * CL, gps)
                    if dc == ND - 1:
                        emit_post(1)
                    else:
                        emit_post()
                else:
                    emit_mul(0)
                    emit_mul(1)
                    # coefficient chain for dc+1 AFTER the muls: its tiny
                    # DVE ops wait on the dc+1 accums, so queueing them
                    # behind the (already-ready) muls avoids head-of-line
                    # blocking, while staying ahead of the tree so the
                    # ScalarE queue still gets grstd/A for the next exps.
                    if dc + 1 < ND:
                        emit_stats_coef(dc + 1)
                        emit_p2(dc + 1)
                    tree_reduce(sesq, eq, gps=gps)
                    emit_post()
                # the s3 half of the epilogue as soon as SE has slack
                if dc == 4:
                    nc.scalar.activation(lns3[:], ps3[:], AF.Ln)
                    nc.scalar.activation(mh3[:], lns3[:], AF.Copy, scale=-0.5)

            # ---------- epilogue ----------
            # sims = s1 / (sqrt(s2)*sqrt(s3)) = s1 * exp(-(ln s2 + ln s3)/2)
            # (l2 eps terms are ~1e-7 relative; dropped). Ln/Exp stay in the
            # one loaded table set. lns3/mh3 were computed mid-loop.
            lns2 = wsp.tile([CL, Bi], F32, tag="lns2")
            nc.scalar.activation(lns2[:], psB[:, Bi:2 * Bi], AF.Ln)
            rden = wsp.tile([CL, Bi], F32, tag="rden")
            nc.scalar.activation(rden[:], lns2[:], AF.Exp, scale=-0.5,
                                 bias=mh3[:])
            sims_sb = smallp.tile([CL, Bi], F32)
            nc.vector.tensor_mul(sims_sb[:], rden[:], psA[:, 0:Bi])
            nc.sync.dma_start(out_ext[:, :], sims_sb[:])

    nc.compile()
    return nc


def _prep_inputs(img_embed, cap_embed, Wg1, bg1, Wg2, bg2, Wb1, bb1, Wb2, bb2,
                 lens):
    """Host-side layout prep + per-core sharding. Returns in_maps (list of 8)."""
    f32 = np.float32
    imgT = np.ascontiguousarray(
        np.transpose(np.asarray(img_embed, f32), (2, 0, 1))).reshape(
            D, BR).astype(BF)
    capf = np.asarray(cap_embed, f32)
    lensf = np.asarray(lens)
    # ragged-mean weights: wfull[c, t] = (t < lens[c]) / lens[c]
    wfull = ((np.arange(T)[None, :] < lensf[:, None]) /
             lensf[:, None].astype(f32)).astype(f32)

    bias_pack = np.concatenate([
        np.asarray(bg1, f32).reshape(H, 1),
        np.asarray(bb1, f32).reshape(H, 1),
        np.asarray(bg2, f32).reshape(ND, P).T,
        np.asarray(bb2, f32).reshape(ND, P).T * GAMMA,
    ], axis=1)
    shared = {
        "imgT": imgT,
        "Wg1": np.ascontiguousarray(np.asarray(Wg1, f32).astype(BF)),
        "Wg2": np.ascontiguousarray(np.asarray(Wg2, f32).astype(BF)),
        "Wb1": np.ascontiguousarray(np.asarray(Wb1, f32).astype(BF)),
        # beta path pre-scaled by GAMMA: the kernel computes iv' = GAMMA*iv,
        # to which sims is invariant (uniform scale cancels in s1/sqrt(s2))
        "Wb2": np.ascontiguousarray((np.asarray(Wb2, f32) * GAMMA).astype(BF)),
        "bias_pack": np.ascontiguousarray(bias_pack),
    }
    in_maps = []
    for i in range(NCORES):
        cs = slice(i * CL, (i + 1) * CL)
        cap_local = np.ascontiguousarray(
            capf[cs].reshape(CL * T, D).astype(BF))
        # block-diagonal mask-weight matrix [(c,t), c']
        wmat = np.zeros((CL * T, CL), f32)
        for cl in range(CL):
            wmat[cl * T:(cl + 1) * T, cl] = wfull[i * CL + cl]
        in_maps.append({**shared, "cap": cap_local,
                        "wm": wmat.astype(BF)})
    return in_maps


def kernel(**inputs) -> np.ndarray:
    global _COMPILED
    from concourse.bass_utils import run_bass_kernel_spmd

    if _COMPILED is None:
        _COMPILED = _build_graph()
    nc = _COMPILED

    in_maps = _prep_inputs(**inputs)
    res = run_bass_kernel_spmd(nc, in_maps, core_ids=list(range(NCORES)))
    sims = np.empty((Bi, Bc), np.float32)
    for i in range(NCORES):
        sims[:, i * CL:(i + 1) * CL] = res.results[i]["out"].T
    return sims


if __name__ == "__main__":
    # smoke test with random data
    rng = np.random.default_rng(0)
    ins = {
        "img_embed": rng.standard_normal((Bi, R, D)).astype(np.float32),
        "cap_embed": rng.standard_normal((Bc, T, D)).astype(np.float32),
        "Wg1": rng.standard_normal((D, H)).astype(np.float32) * 0.02,
        "bg1": np.zeros(H, np.float32),
        "Wg2": rng.standard_normal((H, D)).astype(np.float32) * 0.02,
        "bg2": np.zeros(D, np.float32),
        "Wb1": rng.standard_normal((D, H)).astype(np.float32) * 0.02,
        "bb1": np.zeros(H, np.float32),
        "Wb2": rng.standard_normal((H, D)).astype(np.float32) * 0.02,
        "bb2": np.zeros(D, np.float32),
        "lens": rng.integers(4, T - 4, Bc).astype(np.int32),
    }
    out = kernel(**ins)
    print(out.shape, out.dtype, np.abs(out).mean())
